# revision 7
# baseline (speedup 1.0000x reference)
"""Trainium2 Bass kernel for nn_BioSleepXContrastive (2-layer additive-attention
transformer + top-k memory module). Data-parallel over batch across 8 cores.

Layout strategy per core (2 batches of [T=256, D=80]):
  - h kept as [128, 80] row tiles (i on partitions).
  - Attention: K^T/Q^T as [80(d), 256] so ACT computes tanh(K^T + Q_i) per query
    with Q_i as per-partition bias; PE reduces over d with lhsT=e[:, jblock],
    rhs=u (u = 1/sqrt(D) + wd+ws+wt; the scalar score bias cancels in softmax),
    producing transposed scores [j, i] in PSUM.
  - Softmax without max-subtraction (scores are O(1) bounded: |e|<1, u small):
    exp on ACT straight out of PSUM; Z and attn@V from one matmul with
    rhs=[V | ones].
  - FFN computed transposed: t^T = relu(W1^T @ xn^T + b1) in one matmul + one
    ACT, then out = t^T.T @ W2 per 128-row block.
  - Memory module: top-3 of 64 via DVE Max8 (threshold = 3rd largest),
    softmax weights on masked exp, mem^T = M^T @ p^T via PE.
"""

import os
import sys

import numpy as np

for _p in ("/opt/trn_rl_repo", "/root/.axon_site/_ro/trn_rl_repo"):
    if os.path.isdir(_p) and _p not in sys.path:
        sys.path.append(_p)

D = 80
DFF = 120
T = 256
B = 16
NCORES = 8
BL = B // NCORES  # batches per core
ROWS = BL * T  # 512
MEM = 64
SCALE = float(np.sqrt(D))

_prog_cache = {}


def _build_program():
    from contextlib import ExitStack

    import concourse.bacc as bacc
    import concourse.bass as bass
    import concourse.mybir as mybir
    from concourse.tile import TileContext

    dt = mybir.dt.float32
    AF = mybir.ActivationFunctionType
    ALU = mybir.AluOpType

    nc = bacc.Bacc("TRN2", target_bir_lowering=False, debug=False, num_devices=1)

    def din(name, shape):
        return nc.dram_tensor(name, shape, dt, kind="ExternalInput").ap()

    x_d = din("x", [ROWS, D])
    Wq_d = din("Wq", [2, D, D])
    Wk_d = din("Wk", [2, D, D])
    Wv_d = din("Wv", [2, D, D])
    bq_d = din("bq", [2, D, 1])
    bk_d = din("bk", [2, D, 1])
    bv_d = din("bv", [2, D])
    u_d = din("u", [2, D, 1])
    W1_d = din("W1", [2, D, DFF])
    b1_d = din("b1", [2, DFF, 1])
    W2_d = din("W2", [2, DFF, D])
    b2_d = din("b2", [2, D])
    lnA_d = din("lnA", [4, D])  # [l*2+s, D]
    lnB_d = din("lnB", [4, D])
    na_d = din("na", [D])
    nb_d = din("nb", [D])
    Mt_d = din("Mt", [D, MEM])  # M transposed
    M_d = din("M", [MEM, D])
    Wqm_d = din("Wqm", [D, D])
    bqm_d = din("bqm", [D, 1])
    Wfh_d = din("Wfh", [D, D])  # Wf[:80]
    Wfm_d = din("Wfm", [D, D])  # Wf[80:]
    bf_d = din("bf", [D])
    fg_d = din("fg", [D])
    fb_d = din("fb", [D])
    ident_d = din("ident", [128, 128])

    y_d = nc.dram_tensor("y", [ROWS, D], dt, kind="ExternalOutput").ap()

    def bcast128(src1d):
        # DMA-broadcast a [n]-shaped DRAM AP to [128, n] SBUF via step-0 AP
        return bass.AP(
            tensor=src1d.tensor, offset=src1d.offset, ap=[[0, 128]] + list(src1d.ap)
        )

    with TileContext(nc) as tc, ExitStack() as ctx:
        consts = ctx.enter_context(tc.tile_pool(name="consts", bufs=1))
        hpool = ctx.enter_context(tc.tile_pool(name="hpool", bufs=1))
        work = ctx.enter_context(tc.tile_pool(name="work", bufs=2))
        big = ctx.enter_context(tc.tile_pool(name="big", bufs=2))
        epool = ctx.enter_context(tc.tile_pool(name="epool", bufs=6))
        ps_w = ctx.enter_context(tc.tile_pool(name="ps_w", bufs=2, space="PSUM"))
        ps_sc = ctx.enter_context(tc.tile_pool(name="ps_sc", bufs=1, space="PSUM"))
        ps_o = ctx.enter_context(tc.tile_pool(name="ps_o", bufs=3, space="PSUM"))

        def ctile(shape, tag, src=None, broadcast=False):
            t = consts.tile(shape, dt, tag=tag, name=tag)
            if src is not None:
                if broadcast:
                    nc.gpsimd.dma_start(out=t, in_=bcast128(src))
                else:
                    nc.sync.dma_start(out=t, in_=src)
            return t

        ident = ctile([128, 128], "ident", ident_d)
        Wq_s = [ctile([D, D], f"Wq{l}", Wq_d[l]) for l in range(2)]
        Wk_s = [ctile([D, D], f"Wk{l}", Wk_d[l]) for l in range(2)]
        Wv_s = [ctile([D, D], f"Wv{l}", Wv_d[l]) for l in range(2)]
        bq_s = [ctile([D, 1], f"bq{l}", bq_d[l]) for l in range(2)]
        bk_s = [ctile([D, 1], f"bk{l}", bk_d[l]) for l in range(2)]
        bv_s = [ctile([128, D], f"bv{l}", bv_d[l], broadcast=True) for l in range(2)]
        u_s = [ctile([D, 1], f"u{l}", u_d[l]) for l in range(2)]
        W1_s = [ctile([D, DFF], f"W1{l}", W1_d[l]) for l in range(2)]
        b1_s = [ctile([DFF, 1], f"b1{l}", b1_d[l]) for l in range(2)]
        W2_s = [ctile([DFF, D], f"W2{l}", W2_d[l]) for l in range(2)]
        b2_s = [ctile([128, D], f"b2{l}", b2_d[l], broadcast=True) for l in range(2)]
        lnA_s = [ctile([128, D], f"lnA{i}", lnA_d[i], broadcast=True) for i in range(4)]
        lnB_s = [ctile([128, D], f"lnB{i}", lnB_d[i], broadcast=True) for i in range(4)]
        na_s = ctile([128, D], "na", na_d, broadcast=True)
        nb_s = ctile([128, D], "nb", nb_d, broadcast=True)
        Mt_s = ctile([D, MEM], "Mt", Mt_d)
        M_s = ctile([MEM, D], "M", M_d)
        Wqm_s = ctile([D, D], "Wqm", Wqm_d)
        bqm_s = ctile([D, 1], "bqm", bqm_d)
        Wfh_s = ctile([D, D], "Wfh", Wfh_d)
        Wfm_s = ctile([D, D], "Wfm", Wfm_d)
        bf_s = ctile([128, D], "bf", bf_d, broadcast=True)
        fg_s = ctile([128, D], "fg", fg_d, broadcast=True)
        fb_s = ctile([128, D], "fb", fb_d, broadcast=True)
        eps5 = ctile([128, 1], "eps5")
        nc.vector.memset(eps5, 1e-5)

        # persistent residual-stream tiles, [128, 80] x 4
        h = {}
        for b in range(BL):
            for ib in range(2):
                t = hpool.tile([128, D], dt, tag=f"h{b}{ib}")
                r0 = b * T + ib * 128
                nc.sync.dma_start(out=t, in_=x_d[r0 : r0 + 128, :])
                h[(b, ib)] = t

        def custom_ln(src, a_bc, b_bc, dst_tag):
            """torch-style LN: a*(x-mean)/(sqrt(unbiased var)+1e-6)+b."""
            stats = work.tile([128, 6], dt, tag="lnstats")
            nc.vector.bn_stats(out=stats, in_=src)
            mv = work.tile([128, 2], dt, tag="lnmv")
            nc.vector.bn_aggr(out=mv, in_=stats)
            std = work.tile([128, 1], dt, tag="lnstd")
            # unbiased var = biased var * D/(D-1)
            nc.scalar.activation(
                out=std, in_=mv[:, 1:2], func=AF.Sqrt, scale=float(D) / (D - 1)
            )
            nc.vector.tensor_scalar_add(out=std, in0=std, scalar1=1e-6)
            rinv = work.tile([128, 1], dt, tag="lnrinv")
            nc.vector.reciprocal(out=rinv, in_=std)
            dst = work.tile([128, D], dt, tag=dst_tag)
            nc.vector.tensor_scalar_sub(out=dst, in0=src, scalar1=mv[:, 0:1])
            nc.vector.tensor_scalar_mul(out=dst, in0=dst, scalar1=rinv)
            nc.vector.tensor_mul(out=dst, in0=dst, in1=a_bc)
            nc.vector.tensor_add(out=dst, in0=dst, in1=b_bc)
            return dst

        def transpose2(src0, src1, dst_tag):
            """two [128,80] row tiles -> [80,256] SBUF (PE transpose)."""
            pt = ps_w.tile([D, 256], dt, tag="pw")
            nc.tensor.transpose(out=pt[:, 0:128], in_=src0, identity=ident)
            nc.tensor.transpose(out=pt[:, 128:256], in_=src1, identity=ident)
            dst = big.tile([D, 256], dt, tag=dst_tag)
            nc.vector.tensor_copy(out=dst, in_=pt)
            return dst

        for l in range(2):
            for b in range(BL):
                # ---- attention sublayer ----
                xn0 = custom_ln(h[(b, 0)], lnA_s[2 * l], lnB_s[2 * l], "xn0")
                xn1 = custom_ln(h[(b, 1)], lnA_s[2 * l], lnB_s[2 * l], "xn1")
                xnT = transpose2(xn0, xn1, "xnT")

                qps = ps_w.tile([D, 256], dt, tag="pw")
                nc.tensor.matmul(out=qps, lhsT=Wq_s[l], rhs=xnT, start=True, stop=True)
                qT = big.tile([D, 256], dt, tag="qT")
                nc.vector.tensor_scalar_add(out=qT, in0=qps, scalar1=bq_s[l])

                kps = ps_w.tile([D, 256], dt, tag="pw")
                nc.tensor.matmul(out=kps, lhsT=Wk_s[l], rhs=xnT, start=True, stop=True)
                kT = big.tile([D, 256], dt, tag="kT")
                nc.vector.tensor_scalar_add(out=kT, in0=kps, scalar1=bk_s[l])

                Vo = []
                for jb in range(2):
                    vps = ps_o.tile([128, D], dt, tag="po")
                    nc.tensor.matmul(
                        out=vps,
                        lhsT=xnT[:, jb * 128 : (jb + 1) * 128],
                        rhs=Wv_s[l],
                        start=True,
                        stop=True,
                    )
                    vt = big.tile([128, D + 1], dt, tag=f"Vo{jb}")
                    nc.vector.tensor_add(out=vt[:, 0:D], in0=vps, in1=bv_s[l])
                    nc.vector.memset(vt[:, D : D + 1], 1.0)
                    Vo.append(vt)

                # scores^T [j, i] per j-block, one PSUM column per query
                sc = [
                    ps_sc.tile([128, 256], dt, tag=f"sc{jb}", name=f"sc{jb}")
                    for jb in range(2)
                ]
                for i in range(T):
                    e = epool.tile([D, 256], dt, tag="e")
                    nc.scalar.activation(
                        out=e, in_=kT, func=AF.Tanh, bias=qT[:, i : i + 1]
                    )
                    for jb in range(2):
                        nc.tensor.matmul(
                            out=sc[jb][:, i : i + 1],
                            lhsT=e[:, jb * 128 : (jb + 1) * 128],
                            rhs=u_s[l],
                            start=True,
                            stop=True,
                        )

                PT = []
                for jb in range(2):
                    pt = big.tile([128, 256], dt, tag=f"PT{jb}")
                    nc.scalar.activation(out=pt, in_=sc[jb], func=AF.Exp)
                    PT.append(pt)

                for ib in range(2):
                    ops = ps_o.tile([128, D + 1], dt, tag="po")
                    nc.tensor.matmul(
                        out=ops,
                        lhsT=PT[0][:, ib * 128 : (ib + 1) * 128],
                        rhs=Vo[0],
                        start=True,
                        stop=False,
                    )
                    nc.tensor.matmul(
                        out=ops,
                        lhsT=PT[1][:, ib * 128 : (ib + 1) * 128],
                        rhs=Vo[1],
                        start=False,
                        stop=True,
                    )
                    rz = work.tile([128, 1], dt, tag="rz")
                    nc.vector.reciprocal(out=rz, in_=ops[:, D : D + 1])
                    ot = work.tile([128, D], dt, tag="ot")
                    nc.vector.tensor_scalar_mul(out=ot, in0=ops[:, 0:D], scalar1=rz)
                    nc.vector.tensor_add(out=h[(b, ib)], in0=h[(b, ib)], in1=ot)

                # ---- FFN sublayer ----
                xf0 = custom_ln(h[(b, 0)], lnA_s[2 * l + 1], lnB_s[2 * l + 1], "xn0")
                xf1 = custom_ln(h[(b, 1)], lnA_s[2 * l + 1], lnB_s[2 * l + 1], "xn1")
                xfT = transpose2(xf0, xf1, "xnT")

                tps = ps_w.tile([DFF, 256], dt, tag="pw")
                nc.tensor.matmul(out=tps, lhsT=W1_s[l], rhs=xfT, start=True, stop=True)
                tT = big.tile([DFF, 256], dt, tag="tT")
                nc.scalar.activation(out=tT, in_=tps, func=AF.Relu, bias=b1_s[l])

                for ib in range(2):
                    o2 = ps_o.tile([128, D], dt, tag="po")
                    nc.tensor.matmul(
                        out=o2,
                        lhsT=tT[:, ib * 128 : (ib + 1) * 128],
                        rhs=W2_s[l],
                        start=True,
                        stop=True,
                    )
                    t2 = work.tile([128, D], dt, tag="t2")
                    nc.vector.tensor_add(out=t2, in0=o2, in1=b2_s[l])
                    nc.vector.tensor_add(out=h[(b, ib)], in0=h[(b, ib)], in1=t2)

        # ---- final norm + memory module ----
        for b in range(BL):
            hF = [custom_ln(h[(b, 0)], na_s, nb_s, "hF0"),
                  custom_ln(h[(b, 1)], na_s, nb_s, "hF1")]
            hFT = transpose2(hF[0], hF[1], "hFT")

            qmps = ps_w.tile([D, 256], dt, tag="pw")
            nc.tensor.matmul(out=qmps, lhsT=Wqm_s, rhs=hFT, start=True, stop=True)
            qmT = big.tile([D, 256], dt, tag="qmT")
            nc.vector.tensor_scalar_add(out=qmT, in0=qmps, scalar1=bqm_s)

            for ib in range(2):
                sps = ps_o.tile([128, MEM], dt, tag="po")
                nc.tensor.matmul(
                    out=sps,
                    lhsT=qmT[:, ib * 128 : (ib + 1) * 128],
                    rhs=Mt_s,
                    start=True,
                    stop=True,
                )
                sim = work.tile([128, MEM], dt, tag="sim")
                nc.vector.tensor_copy(out=sim, in_=sps)
                mx8 = work.tile([128, 8], dt, tag="mx8")
                nc.vector.max(out=mx8, in_=sim)
                negm = work.tile([128, 1], dt, tag="negm")
                nc.scalar.mul(out=negm, in_=mx8[:, 0:1], mul=-1.0)
                p = work.tile([128, MEM], dt, tag="p")
                nc.scalar.activation(out=p, in_=sim, func=AF.Exp, bias=negm)
                msk = work.tile([128, MEM], dt, tag="msk")
                nc.vector.tensor_scalar(
                    out=msk, in0=sim, scalar1=mx8[:, 2:3], scalar2=None, op0=ALU.is_ge
                )
                nc.vector.tensor_mul(out=p, in0=p, in1=msk)
                zs = work.tile([128, 1], dt, tag="zs")
                nc.vector.tensor_reduce(
                    out=zs, in_=p, axis=mybir.AxisListType.X, op=ALU.add
                )
                nc.vector.reciprocal(out=zs, in_=zs)
                nc.vector.tensor_scalar_mul(out=p, in0=p, scalar1=zs)

                pTps = ps_o.tile([MEM, 128], dt, tag="po")
                nc.tensor.transpose(out=pTps, in_=p, identity=ident)
                pT = work.tile([MEM, 128], dt, tag="pT")
                nc.vector.tensor_copy(out=pT, in_=pTps)

                mTps = ps_o.tile([D, 128], dt, tag="po")
                nc.tensor.matmul(out=mTps, lhsT=M_s, rhs=pT, start=True, stop=True)
                memT = work.tile([D, 128], dt, tag="memT")
                nc.vector.tensor_copy(out=memT, in_=mTps)

                fps = ps_o.tile([128, D], dt, tag="po")
                nc.tensor.matmul(
                    out=fps,
                    lhsT=hFT[:, ib * 128 : (ib + 1) * 128],
                    rhs=Wfh_s,
                    start=True,
                    stop=False,
                )
                nc.tensor.matmul(
                    out=fps, lhsT=memT, rhs=Wfm_s, start=False, stop=True
                )
                fo = work.tile([128, D], dt, tag="fo")
                nc.vector.tensor_add(out=fo, in0=fps, in1=bf_s)

                # std LN (biased var, eps inside sqrt) + relu
                stats = work.tile([128, 6], dt, tag="lnstats")
                nc.vector.bn_stats(out=stats, in_=fo)
                mv = work.tile([128, 2], dt, tag="lnmv")
                nc.vector.bn_aggr(out=mv, in_=stats)
                std = work.tile([128, 1], dt, tag="lnstd")
                nc.scalar.activation(out=std, in_=mv[:, 1:2], func=AF.Sqrt, bias=eps5)
                rinv = work.tile([128, 1], dt, tag="lnrinv")
                nc.vector.reciprocal(out=rinv, in_=std)
                on = work.tile([128, D], dt, tag="on")
                nc.vector.tensor_scalar_sub(out=on, in0=fo, scalar1=mv[:, 0:1])
                nc.vector.tensor_scalar_mul(out=on, in0=on, scalar1=rinv)
                nc.vector.tensor_mul(out=on, in0=on, in1=fg_s)
                nc.vector.tensor_add(out=on, in0=on, in1=fb_s)
                nc.vector.tensor_scalar_max(out=on, in0=on, scalar1=0.0)

                r0 = b * T + ib * 128
                nc.sync.dma_start(out=y_d[r0 : r0 + 128, :], in_=on)

    nc.compile()
    return nc


def _get_nc():
    if "nc" not in _prog_cache:
        _prog_cache["nc"] = _build_program()
    return _prog_cache["nc"]


def kernel(**inputs):
    from concourse.bass_utils import run_bass_kernel_spmd

    nc = _get_nc()
    f32 = np.float32

    def a(name):
        return np.ascontiguousarray(np.asarray(inputs[name], dtype=f32))

    x = a("x")  # [16, 256, 80]
    wd, ws, wt = a("wd"), a("ws"), a("wt")
    u = (wd + ws + wt + 1.0 / SCALE).reshape(2, D, 1).astype(f32)
    ln_a, ln_b = a("ln_a"), a("ln_b")
    Wf = a("Wf")
    base = {
        "Wq": a("Wq"),
        "Wk": a("Wk"),
        "Wv": a("Wv"),
        "bq": a("bq").reshape(2, D, 1),
        "bk": a("bk").reshape(2, D, 1),
        "bv": a("bv"),
        "u": u,
        "W1": a("W1"),
        "b1": a("b1").reshape(2, DFF, 1),
        "W2": a("W2"),
        "b2": a("b2"),
        "lnA": ln_a.reshape(4, D),
        "lnB": ln_b.reshape(4, D),
        "na": a("na"),
        "nb": a("nb"),
        "Mt": np.ascontiguousarray(a("M").T),
        "M": a("M"),
        "Wqm": a("Wqm"),
        "bqm": a("bqm").reshape(D, 1),
        "Wfh": np.ascontiguousarray(Wf[:D]),
        "Wfm": np.ascontiguousarray(Wf[D:]),
        "bf": a("bf"),
        "fg": a("fg"),
        "fb": a("fb"),
        "ident": np.eye(128, dtype=f32),
    }
    in_maps = []
    for c in range(NCORES):
        m = dict(base)
        m["x"] = np.ascontiguousarray(
            x[BL * c : BL * (c + 1)].reshape(ROWS, D)
        )
        in_maps.append(m)

    r = run_bass_kernel_spmd(nc, in_maps, core_ids=list(range(NCORES)))
    out = np.stack([r.results[c]["y"] for c in range(NCORES)])
    return out.reshape(B, T, D).astype(f32)
